# revision 36
# baseline (speedup 1.0000x reference)
"""Trainium2 Bass kernel for nn_L2_Self_Attn_Old (B=4, C=128, H=W=64, N=4096).

Strategy (8 cores = 4 batches x 2 sequence-halves):
  Core (b, h) computes att rows [2048h, 2048(h+1)) of batch b via a
  transposed flash softmax (no max-tracking needed: true L2 logits <= 0;
  the n-dependent score term cancels in softmax normalization, the
  m-dependent term is a per-partition ACT bias), then pushes its half
  through the (linear) epilogue with the other half zeroed. Host sums the
  two partials per batch (partial-sum unshard).

Raw-reshape identities used for the on-device (N,C)<->(C,N) reinterprets
(via DRAM bounce with affine APs):  Att_cn[c', 128q+r] = att[32c'+q, r].
"""

import os
import numpy as np
import ml_dtypes

_OPT = lambda k, d: int(os.environ.get(k, d))

import concourse.bass as bass
import concourse.mybir as mybir
import concourse.tile as tile
from concourse import bacc
from concourse.bass_utils import run_bass_kernel_spmd

F32 = mybir.dt.float32
BF16 = mybir.dt.bfloat16
BF = ml_dtypes.bfloat16

C = 128
N = 4096
NSH = N // 2          # 2048 rows per core
S1M = 2.0 / np.sqrt(np.float32(C))       # -2*scalar1 (positive)
BIAS_SCALE = -0.5 * float(S1M)            # multiplies nq[m]
CHUNK = 1024                              # flash n-chunk
NCHUNKS = NSH // CHUNK                    # 2
MT = N // 128                             # 32 m-tiles

_cache = {}


def _lambertw_real(z):
    w = np.log(z) - np.log(np.log(z))
    for _ in range(20):
        ew = np.exp(w)
        w = w - (w * ew - z) / (ew * (w + 1.0))
    return float(w)


def _build_nc():
    nc = bacc.Bacc(None)
    xcn = nc.dram_tensor("xcn", [C, N], BF16, kind="ExternalInput")
    xnc = nc.dram_tensor("xnc", [C, N], BF16, kind="ExternalInput")
    pre = nc.dram_tensor("pre", [C, N], F32, kind="ExternalInput")
    wqT = nc.dram_tensor("wqT", [C, C], BF16, kind="ExternalInput")
    wqh = nc.dram_tensor("wqh", [64, C], BF16, kind="ExternalInput")
    wq2T = nc.dram_tensor("wq2T", [C, C], BF16, kind="ExternalInput")
    wvT = nc.dram_tensor("wvT", [C, C], BF16, kind="ExternalInput")
    bqc = nc.dram_tensor("bqc", [C, 1], F32, kind="ExternalInput")
    bqe = nc.dram_tensor("bqe", [C, 1], F32, kind="ExternalInput")
    ident = nc.dram_tensor("ident", [C, C], BF16, kind="ExternalInput")
    out = nc.dram_tensor("out", [C, N], F32, kind="ExternalOutput")

    EXP = mybir.ActivationFunctionType.Exp
    MUL = mybir.AluOpType.mult

    with tile.TileContext(nc) as tc:
        with (
            tc.tile_pool(name="big", bufs=1) as big,        # long-lived sbuf
            tc.tile_pool(name="wpool", bufs=_OPT("KOPT_WBUFS", 6)) as wpool,
            tc.tile_pool(name="mid", bufs=2) as mid,
            tc.tile_pool(name="pg", bufs=2, space="PSUM") as pg,    # 2x2 banks
            tc.tile_pool(name="pa", bufs=1, space="PSUM") as pa,    # 2 banks
            tc.tile_pool(name="px", bufs=2, space="PSUM") as px,    # 2x1 banks
            tc.tile_pool(name="dpool", bufs=1, space="DRAM") as dpool,
        ):
            # ---------------- load ----------------
            sb_xcn = big.tile([C, N], BF16, tag="xcn")
            sb_xnc = big.tile([C, N], BF16, tag="xnc")
            sb_wqT = big.tile([C, C], BF16, tag="wqT")
            sb_wqh = big.tile([64, C], BF16, tag="wqh")
            sb_wq2T = big.tile([C, C], BF16, tag="wq2T")
            sb_wvT = big.tile([C, C], BF16, tag="wvT")
            sb_bqc = big.tile([C, 1], F32, tag="bqc")
            sb_bqe = big.tile([C, 1], F32, tag="bqe")
            sb_id = big.tile([C, C], BF16, tag="ident")
            nc.sync.dma_start(sb_wqT[:], wqT[:])
            nc.sync.dma_start(sb_bqc[:], bqc[:])
            nc.sync.dma_start(sb_xcn[:, 0:2048], xcn[:, 0:2048])
            nc.sync.dma_start(sb_xcn[:, 2048:4096], xcn[:, 2048:4096])
            nc.scalar.dma_start(sb_xnc[:], xnc[:])
            nc.gpsimd.dma_start(sb_id[:], ident[:])
            nc.gpsimd.dma_start(sb_wqh[:], wqh[:])
            sb_pre = big.tile([C, N], F32, tag="pre")
            nc.gpsimd.dma_start(sb_pre[:], pre[:])
            nc.gpsimd.dma_start(sb_wq2T[:], wq2T[:])
            nc.gpsimd.dma_start(sb_wvT[:], wvT[:])
            nc.gpsimd.dma_start(sb_bqe[:], bqe[:])

            ones_bf = big.tile([C, 1], BF16, tag="ones")
            nc.vector.memset(ones_bf[:], 1.0)

            # ---------------- Q = Wq @ X_cn + bq; nq per m-tile (pipelined) ----
            sb_q = big.tile([C, N], BF16, tag="q")
            sb_qsq = big.tile([C, N], BF16, tag="qsq")
            ps_nq = px.tile([C, MT], F32, tag="px")
            for j in range(8):
                ps_q = pg.tile([C, 512], F32, tag="pg")
                nc.tensor.matmul(ps_q[:], sb_wqT[:], sb_xcn[:, j * 512:(j + 1) * 512],
                                 start=True, stop=True)
                if _OPT("KOPT_QBIAS_DVE", 0):
                    nc.vector.tensor_scalar_add(sb_q[:, j * 512:(j + 1) * 512],
                                                ps_q[:], sb_bqc[:])
                else:
                    nc.scalar.add(sb_q[:, j * 512:(j + 1) * 512], ps_q[:], sb_bqc[:])
                nc.vector.tensor_mul(sb_qsq[:, j * 512:(j + 1) * 512],
                                     sb_q[:, j * 512:(j + 1) * 512],
                                     sb_q[:, j * 512:(j + 1) * 512])
            sb_bias = big.tile([C, MT], F32, tag="bias")
            for j in range(8):
                for t in range(4 * j, 4 * j + 4):
                    nc.tensor.matmul(ps_nq[:, t:t + 1],
                                     sb_qsq[:, t * 128:(t + 1) * 128],
                                     ones_bf[:], start=(t == 0), stop=(t == MT - 1),
                                     skip_group_check=True)
                # per-j bias slice so early exps don't wait on all of nq
                nc.scalar.mul(sb_bias[:, 4 * j:4 * j + 4],
                              ps_nq[:, 4 * j:4 * j + 4], BIAS_SCALE)

            # ---------------- flash ----------------
            sb_att = big.tile([C, NSH], BF16, tag="att")   # (n%128 part, tile-major)
            dram_att = dpool.tile([NSH, C], BF16, tag="datt")
            sb_attcn = big.tile([64, N], BF16, tag="attcn")
            sb_ah1 = big.tile([C, N], BF16, tag="ah1")     # A_half part-1 (K=32)
            for ch in range(NCHUNKS):
                base = ch * CHUNK
                ps_att = pa.tile([C, CHUNK], F32, tag="pa")
                sb_acc = mid.tile([C, CHUNK], BF16, tag="acc")
                for mi in range(MT):
                    ps_g = pg.tile([C, CHUNK], F32, tag="pg")
                    qm = sb_q[:, mi * 128:(mi + 1) * 128]
                    nc.tensor.matmul(ps_g[:, 0:512], qm,
                                     sb_q[:, base:base + 512], start=True, stop=True)
                    nc.tensor.matmul(ps_g[:, 512:1024], qm,
                                     sb_q[:, base + 512:base + 1024],
                                     start=True, stop=True)
                    w_t = wpool.tile([C, CHUNK], BF16, tag="w")
                    nc.scalar.activation(w_t[:], ps_g[:], EXP,
                                         bias=sb_bias[:, mi:mi + 1], scale=float(S1M))
                    xm = sb_xnc[:, mi * 128:(mi + 1) * 128]
                    nc.tensor.matmul(ps_att[:, 0:512], xm, w_t[:, 0:512],
                                     start=(mi == 0), stop=(mi == MT - 1))
                    nc.tensor.matmul(ps_att[:, 512:1024], xm, w_t[:, 512:1024],
                                     start=(mi == 0), stop=(mi == MT - 1))
                    if mi == 0:
                        nc.vector.tensor_copy(sb_acc[:], w_t[:])
                    elif _OPT("KOPT_ACC_GPS", 0) and mi % 4 == 3:
                        nc.gpsimd.tensor_add(sb_acc[:], sb_acc[:], w_t[:])
                    else:
                        nc.vector.tensor_add(sb_acc[:], sb_acc[:], w_t[:])

                # S columns (transposed via tiny matmuls) + reciprocal
                ps_s = px.tile([C, 8], F32, tag="px")
                for v in range(8):
                    nc.tensor.matmul(ps_s[:, v:v + 1], sb_acc[:, v * 128:(v + 1) * 128],
                                     ones_bf[:], start=(v == 0), stop=(v == 7),
                                     skip_group_check=True)
                sb_invs = mid.tile([C, 8], F32, tag="invs")
                nc.vector.reciprocal(sb_invs[:], ps_s[:])

                # evacuate att chunk, transpose 128-blocks, scale by 1/S
                sb_awT = mid.tile([C, CHUNK], BF16, tag="awT")
                nc.vector.tensor_copy(sb_awT[:], ps_att[:])
                for v in range(8):
                    ps_t = px.tile([C, C], BF16, tag="px")
                    nc.tensor.transpose(ps_t[:], sb_awT[:, v * 128:(v + 1) * 128],
                                        sb_id[:])
                    nc.vector.tensor_scalar(
                        out=sb_att[:, base + v * 128:base + (v + 1) * 128],
                        in0=ps_t[:], scalar1=sb_invs[:, v:v + 1], scalar2=None,
                        op0=MUL)

                if _OPT("KOPT_R0_DIRECT", 0):
                    # direct sbuf->sbuf flatten: row r <- att tile r//4 slice
                    engs = [nc.sync, nc.scalar, nc.gpsimd]
                    for rr in range(32):
                        r = 32 * ch + rr
                        vloc = r // 4 - 8 * ch
                        engs[rr % 3].dma_start(
                            sb_attcn[r:r + 1, :].rearrange("p (q c) -> p q c", c=128),
                            sb_att[32 * (r % 4):32 * (r % 4) + 32,
                                   base + 128 * vloc:base + 128 * vloc + 128])
                else:
                    # R0 write for this chunk (overlaps next chunk's compute)
                    for gg in range(2):
                        g = 2 * ch + gg
                        eng = nc.sync if gg == 0 else nc.scalar
                        eng.dma_start(
                            dram_att[512 * g:512 * (g + 1), :]
                            .rearrange("(v p) c -> p v c", p=128),
                            sb_att[:, 512 * g:512 * (g + 1)]
                            .rearrange("p (v c) -> p v c", c=128))
                    # R0 read: rows [32ch, 32ch+32) from this chunk's tiles
                    eng = nc.sync if ch == 0 else nc.gpsimd
                    eng.dma_start(
                        sb_attcn[32 * ch:32 * (ch + 1), :],
                        dram_att[:].rearrange("(r q) c -> r (q c)", q=32)
                        [32 * ch:32 * (ch + 1), :])

                # A_half part 1 (K=32 contraction over attcn rows 0:32)
                # overlaps flash chunk 1; uses px-pool psums (free mid-chunk).
                if ch == 0:
                    for j in range(8):
                        ps_ah = px.tile([C, 512], F32, tag="px")
                        nc.tensor.matmul(
                            ps_ah[:], sb_wqh[0:32, :],
                            sb_attcn[0:32, j * 512:(j + 1) * 512],
                            start=True, stop=True)
                        nc.vector.tensor_copy(sb_ah1[:, j * 512:(j + 1) * 512],
                                              ps_ah[:])

            # ---------------- epilogue (partial: this half only) ----------------
            # A_half part 2 (K=32 over attcn rows 32:64) + combine with part 1;
            # R1 write per column-chunk right after each evac (pipelines).
            sb_ah = big.tile([C, N], BF16, tag="ah")
            dram_ah = dpool.tile([C, N], BF16, tag="dah")
            dram_a2 = dpool.tile([N, C], BF16, tag="da2")
            sb_ahn = big.tile([C, N], BF16, tag="ahn")
            sb_att2 = big.tile([C, N], BF16, tag="att2")
            sb_a2cn = big.tile([C, N], BF16, tag="a2cn")
            ah_nc_view = dram_ah[:].rearrange(
                "(t ph) (pl k) -> (ph pl) t k", ph=4, k=128)
            a2cn_view = dram_a2[:].rearrange("(c q) r -> c (q r)", q=32)
            for j in range(8):
                ps_ah = px.tile([C, 512], F32, tag="px")
                nc.tensor.matmul(ps_ah[:], sb_wqh[32:64, :],
                                 sb_attcn[32:64, j * 512:(j + 1) * 512],
                                 start=True, stop=True)
                nc.vector.scalar_tensor_tensor(
                    out=sb_ah[:, j * 512:(j + 1) * 512],
                    in0=ps_ah[:], scalar=sb_bqe[:],
                    in1=sb_ah1[:, j * 512:(j + 1) * 512],
                    op0=mybir.AluOpType.add, op1=mybir.AluOpType.add)
                if _OPT("KOPT_R1W_COL", 1):
                    nc.gpsimd.dma_start(dram_ah[:, j * 512:(j + 1) * 512],
                                        sb_ah[:, j * 512:(j + 1) * 512])
            if not _OPT("KOPT_R1W_COL", 1):
                # per-row-group writes: R1 read group g then only waits write g
                for g in range(4):
                    nc.gpsimd.dma_start(dram_ah[32 * g:32 * (g + 1), :],
                                        sb_ah[32 * g:32 * (g + 1), :])

            # pipelined groups g: R1 read -> transpose/matmul -> R2 bounce
            for g in range(4):
                eng0 = nc.sync if g % 2 == 0 else nc.scalar
                eng1 = nc.scalar if g % 2 == 0 else nc.sync
                # R1 read tiles t in [8g, 8g+8)
                eng1.dma_start(
                    sb_ahn[:, 1024 * g:1024 * (g + 1)]
                    .rearrange("p (t k) -> p t k", k=128),
                    ah_nc_view[:, 8 * g:8 * (g + 1), :])
                # transposes + att2 matmuls for this group's 8 tiles
                for half in range(2):
                    ps_tt = px.tile([C, 512], BF16, tag="px")
                    for j in range(4):
                        t = 8 * g + 4 * half + j
                        nc.tensor.transpose(ps_tt[:, j * 128:(j + 1) * 128],
                                            sb_ahn[:, t * 128:(t + 1) * 128], sb_id[:])
                    sb_ahT = mid.tile([C, 512], BF16, tag="ahT")
                    nc.vector.tensor_copy(sb_ahT[:], ps_tt[:])
                    ps_a2 = pg.tile([C, 512], F32, tag="pg")
                    for j in range(4):
                        nc.tensor.matmul(ps_a2[:, j * 128:(j + 1) * 128],
                                         sb_ahT[:, j * 128:(j + 1) * 128], sb_wq2T[:],
                                         start=(j == 0), stop=(j == 3),
                                         skip_group_check=True)
                    o = 1024 * g + 512 * half
                    if half == 0:
                        nc.vector.tensor_copy(sb_att2[:, o:o + 512], ps_a2[:])
                    else:
                        nc.scalar.copy(sb_att2[:, o:o + 512], ps_a2[:])
                if _OPT("KOPT_R2_DIRECT", 0):
                    # direct sbuf->sbuf flatten per tile t
                    engs = [nc.sync, nc.scalar, nc.gpsimd]
                    for tt in range(8):
                        t = 8 * g + tt
                        engs[tt % 3].dma_start(
                            sb_a2cn[4 * t:4 * t + 4, :]
                            .rearrange("p (b k) -> p b k", k=128),
                            sb_att2[:, t * 128:(t + 1) * 128])
                else:
                    # R2 write tiles [8g, 8g+8), read rows [32g, 32g+32)
                    eng0.dma_start(
                        dram_a2[1024 * g:1024 * (g + 1), :]
                        .rearrange("(t p) j -> p t j", p=128),
                        sb_att2[:, 1024 * g:1024 * (g + 1)]
                        .rearrange("p (t j) -> p t j", j=128))
                    nc.gpsimd.dma_start(sb_a2cn[32 * g:32 * (g + 1), :],
                                        a2cn_view[32 * g:32 * (g + 1), :])

            # out = wvT.T @ Att2_cn + pre ; DMA out
            sb_out = big.tile([C, N], F32, tag="out")
            for j in range(8):
                ps_o = pg.tile([C, 512], F32, tag="pg")
                nc.tensor.matmul(ps_o[:], sb_wvT[:], sb_a2cn[:, j * 512:(j + 1) * 512],
                                 start=True, stop=True)
                nc.vector.tensor_add(sb_out[:, j * 512:(j + 1) * 512], ps_o[:],
                                     sb_pre[:, j * 512:(j + 1) * 512])
                eng = nc.sync if j % 2 == 0 else nc.scalar
                eng.dma_start(out[:, j * 512:(j + 1) * 512],
                              sb_out[:, j * 512:(j + 1) * 512])

    nc.compile()
    return nc


def kernel(x, Wq, bq, Wv, bv, gamma):
    x = np.ascontiguousarray(np.asarray(x, dtype=np.float32))
    Wq = np.asarray(Wq, np.float32)
    bq = np.asarray(bq, np.float32)
    Wv = np.asarray(Wv, np.float32)
    bv = np.asarray(bv, np.float32)
    gamma = np.asarray(gamma, np.float32)
    B = x.shape[0]

    if "nc" not in _cache:
        _cache["nc"] = _build_nc()
    nc = _cache["nc"]

    phi = _lambertw_real(N / np.e)
    bound = (np.sqrt(np.float32(N / C)) * np.float32(4.0 * phi + 1.0)
             * np.linalg.norm(Wq) * np.linalg.norm(Wv))
    gb = np.float32(gamma[0] / bound)
    s2 = np.float32(1.0 / np.sqrt(np.float32(C)))

    wqT = np.ascontiguousarray(Wq.T).astype(BF)
    wq2T = np.ascontiguousarray((s2 * Wq).T).astype(BF)
    wvT = np.ascontiguousarray((gb * Wv).T).astype(BF)
    bqc = bq.reshape(C, 1).astype(np.float32)
    idm = np.eye(C, dtype=BF)
    zeros_col = np.zeros((C, 1), np.float32)

    in_maps = []
    for core in range(8):
        b, h = core // 2, core % 2
        X_cn = x[b].reshape(C, N)
        X_nc = x[b].reshape(N, C)
        xnc_prep = np.ascontiguousarray(
            X_nc.reshape(MT, 128, C).transpose(1, 0, 2).reshape(C, N)).astype(BF)
        if h == 0:
            pre = (X_cn + gb * bv[:, None]).astype(np.float32)
            bqe = bqc
        else:
            pre = np.zeros((C, N), np.float32)
            bqe = zeros_col
        in_maps.append({
            "xcn": X_cn.astype(BF),
            "xnc": xnc_prep,
            "pre": pre,
            "wqT": wqT,
            "wqh": np.ascontiguousarray(Wq.T[64 * h:64 * h + 64]).astype(BF),
            "wq2T": wq2T,
            "wvT": wvT,
            "bqc": bqc,
            "bqe": bqe,
            "ident": idm,
        })

    res = run_bass_kernel_spmd(nc, in_maps, core_ids=list(range(8)))
    kernel._last_result = res

    out = np.empty((B, C, 64, 64), np.float32)
    for b in range(B):
        s = res.results[2 * b]["out"] + res.results[2 * b + 1]["out"]
        out[b] = s.reshape(C, 64, 64)
    return out
